# revision 1
# baseline (speedup 1.0000x reference)
"""Trainium2 Bass kernel for per-object 3-layer MLP (grouped GEMV).

Problem: for each of 2048 objects o (each with private weights):
    y1 = W1[o] @ x[o] + b1[o]                  # [256]
    y2 = sigmoid(W2[o] @ y1 + b2[o])           # [256]
    y3 = sigmoid(W3[o] @ y2 + b3[o])[0]        # scalar
Memory-bound: ~771 MB of weights total, ~96.5 MB per core across 8 cores.

Strategy (per core, 256 objects):
  - objects on SBUF partitions, 128 per block (2 blocks per core)
  - weights DMA'd in large contiguous chunks (objects x m-range x contraction)
  - per output feature: one fused DVE tensor_tensor_reduce
    (product along free axis, accumulate-reduce seeded with the bias)
  - sigmoid on the scalar engine
"""

import contextlib

import numpy as np

import bass_rust
import concourse.bass as bass
import concourse.mybir as mybir
import concourse.tile as tile
from concourse.bass_utils import run_bass_kernel_spmd
from concourse.vector_clock import ScopedClock

# ---------------------------------------------------------------------------
# This container's walrus build supports only ONE sync-wait per instruction.
# Tile's scheduler can attach several; split the extras onto standalone nops
# placed immediately before the instruction (same engine), and do the same
# for the kernel-tail drain.
# ---------------------------------------------------------------------------

_ORIG_LOWER = tile.TileContext._lower_ordered_insts


def _split_multi_waits(ordered):
    for bb_name, insts in ordered.items():
        needs_split = any(
            getattr(i, "sync_info", None) is not None
            and len(i.sync_info.on_wait) > 1
            for i in insts
        )
        if not needs_split:
            continue
        new = []
        for inst in insts:
            si = getattr(inst, "sync_info", None)
            eng = getattr(inst, "engine", None)
            if si is not None and len(si.on_wait) > 1 and eng is not None:
                waits = list(si.on_wait)
                si.on_wait = waits[-1:]
                for k, w in enumerate(waits[:-1]):
                    new.append(mybir.InstNoOp(
                        name=f"{inst.name}_wsplit{k}",
                        sync_info=mybir.SyncInfo(on_wait=[w], on_update=[]),
                        bass_nofuse=True,
                        engine=eng,
                    ))
            new.append(inst)
        insts[:] = new


def _patched_lower(self, ordered):
    _split_multi_waits(ordered)
    return _ORIG_LOWER(self, ordered)


def _patched_drain_and_barrier(self, tick_clock, wait_clock):
    drain_inst = self.nc.sync.drain()
    wait_clock.add_sem_waits(
        drain_inst.ins, ScopedClock({None: tick_clock.global_clock})
    )
    si = drain_inst.ins.sync_info
    if si is not None and len(si.on_wait) > 1:
        waits = list(si.on_wait)
        si.on_wait = waits[:1]
        for w in waits[1:]:
            n = self.nc.sync.nop(nofuse=True)
            n.ins.sync_info = bass_rust.SyncInfo(on_wait=[w], on_update=[])

    self.nc.all_engine_barrier()
    assert self.sems is not None
    popped = self.nc._tile_sem_poison_stack.pop()
    assert popped is self._sem_poison
    self.nc.clear_and_free_semaphores(list(self.sems.allocated().values()))
    self.nc.all_engine_barrier()


tile.TileContext._lower_ordered_insts = _patched_lower
tile.TileContext._drain_and_barrier = _patched_drain_and_barrier

N_CORES = 8
N_OBJ = 2048
O_PER_CORE = N_OBJ // N_CORES  # 256
BLK = 128                      # objects per block = SBUF partitions
N_BLK = O_PER_CORE // BLK      # 2
IN_DIM = 128
MID = 256
M_CHUNK = 64                   # L1: m-values per W1 DMA chunk (32 KiB/partition)
N_CHUNK = 32                   # L2: n-values per W2 DMA chunk (32 KiB/partition)

F32 = mybir.dt.float32
_nullctx = contextlib.nullcontext


SUB1 = 16   # L1 m-values per big TT multiply (FD = 16*128 = 2048)
SUB2 = 8    # L2 n-values per big TT multiply (FD = 8*256 = 2048)


def build_bass_split(repeats: int = 1) -> bass.Bass:
    """Strategy 'split': big-FD TT multiplies + segmented DVE reduce (L1),
    per-feature ACT accum reduce (L2). All fp32."""
    nc = bass.Bass("TRN2", target_bir_lowering=False, debug=False,
                   num_devices=N_CORES)

    x_d = nc.dram_tensor("x", [O_PER_CORE, IN_DIM], F32, kind="ExternalInput").ap()
    w1_d = nc.dram_tensor("W1", [O_PER_CORE, MID, IN_DIM], F32, kind="ExternalInput").ap()
    b1_d = nc.dram_tensor("b1", [O_PER_CORE, MID], F32, kind="ExternalInput").ap()
    w2_d = nc.dram_tensor("W2", [O_PER_CORE, MID, MID], F32, kind="ExternalInput").ap()
    b2_d = nc.dram_tensor("b2", [O_PER_CORE, MID], F32, kind="ExternalInput").ap()
    w3_d = nc.dram_tensor("W3", [O_PER_CORE, 1, MID], F32, kind="ExternalInput").ap()
    b3_d = nc.dram_tensor("b3", [O_PER_CORE, 1], F32, kind="ExternalInput").ap()
    y_d = nc.dram_tensor("y", [O_PER_CORE], F32, kind="ExternalOutput").ap()

    with tile.TileContext(nc) as tc:
        with (
            tc.tile_pool(name="wpool", bufs=2) as wpool,
            tc.tile_pool(name="ppool", bufs=2) as ppool,
            tc.tile_pool(name="apool", bufs=2) as apool,
            tc.tile_pool(name="rpool", bufs=1) as rpool,
        ):
          with (tc.For_i(0, repeats, 1) if repeats > 1 else _nullctx()):
            for b in range(N_BLK):
                osl = slice(b * BLK, (b + 1) * BLK)

                xt = apool.tile([BLK, IN_DIM], F32, name="xt")
                nc.sync.dma_start(out=xt, in_=x_d[osl, :])
                b1t = apool.tile([BLK, MID], F32, name="b1t")
                nc.sync.dma_start(out=b1t, in_=b1_d[osl, :])
                b2t = apool.tile([BLK, MID], F32, name="b2t")
                nc.sync.dma_start(out=b2t, in_=b2_d[osl, :])
                b3t = apool.tile([BLK, 1], F32, name="b3t")
                nc.sync.dma_start(out=b3t, in_=b3_d[osl, :])
                w3t = apool.tile([BLK, MID], F32, name="w3t")
                nc.sync.dma_start(out=w3t, in_=w3_d[osl, 0, :])

                y1p = apool.tile([BLK, MID], F32, name="y1p")
                y1 = apool.tile([BLK, MID], F32, name="y1")
                y2p = apool.tile([BLK, MID], F32, name="y2p")
                y2 = apool.tile([BLK, MID], F32, name="y2")

                # xrep: x repeated SUB1 times along free
                xrep = rpool.tile([BLK, SUB1, IN_DIM], F32, name="xrep")
                for s in range(SUB1):
                    nc.vector.tensor_copy(xrep[:, s, :], xt)

                # ---- layer 1 ----
                for mc in range(MID // M_CHUNK):
                    w1c = wpool.tile([BLK, M_CHUNK, IN_DIM], F32, name="w1c",
                                     tag="w1c")
                    nc.sync.dma_start(
                        out=w1c,
                        in_=w1_d[osl, mc * M_CHUNK:(mc + 1) * M_CHUNK, :])
                    for s in range(M_CHUNK // SUB1):
                        m0 = s * SUB1
                        t1 = ppool.tile([BLK, SUB1, IN_DIM], F32, name="t1",
                                        tag="t1")
                        nc.vector.tensor_mul(
                            out=t1, in0=w1c[:, m0:m0 + SUB1, :], in1=xrep)
                        nc.vector.tensor_reduce(
                            out=y1p[:, mc * M_CHUNK + m0:
                                    mc * M_CHUNK + m0 + SUB1],
                            in_=t1,
                            axis=mybir.AxisListType.X,
                            op=mybir.AluOpType.add)
                nc.vector.tensor_add(out=y1, in0=y1p, in1=b1t)

                # y1rep: y1 repeated SUB2 times along free
                y1rep = rpool.tile([BLK, SUB2, MID], F32, name="y1rep")
                for s in range(SUB2):
                    nc.vector.tensor_copy(y1rep[:, s, :], y1)

                # ---- layer 2 ----
                for nc_i in range(MID // N_CHUNK):
                    w2c = wpool.tile([BLK, N_CHUNK, MID], F32, name="w2c",
                                     tag="w2c")
                    nc.sync.dma_start(
                        out=w2c,
                        in_=w2_d[osl, nc_i * N_CHUNK:(nc_i + 1) * N_CHUNK, :])
                    for s in range(N_CHUNK // SUB2):
                        n0 = s * SUB2
                        t2 = ppool.tile([BLK, SUB2, MID], F32, name="t2",
                                        tag="t2")
                        nc.vector.tensor_mul(
                            out=t2, in0=w2c[:, n0:n0 + SUB2, :], in1=y1rep)
                        scr = ppool.tile([BLK, MID], F32, name="scr",
                                         tag="scr")
                        for j in range(SUB2):
                            nn = nc_i * N_CHUNK + n0 + j
                            nc.scalar.activation(
                                out=scr,
                                in_=t2[:, j, :],
                                func=mybir.ActivationFunctionType.Copy,
                                accum_out=y2p[:, nn:nn + 1])
                nc.vector.tensor_add(out=y2p, in0=y2p, in1=b2t)
                nc.scalar.activation(out=y2, in_=y2p,
                                     func=mybir.ActivationFunctionType.Sigmoid)

                # ---- layer 3 ----
                dum3 = apool.tile([BLK, 1], F32, name="dum3")
                y3p = apool.tile([BLK, 1], F32, name="y3p")
                nc.vector.scalar_tensor_tensor(
                    out=dum3.broadcast_to((BLK, MID)),
                    in0=w3t,
                    scalar=1.0,
                    in1=y2,
                    op0=mybir.AluOpType.mult,
                    op1=mybir.AluOpType.mult,
                    accum_out=y3p,
                )
                nc.vector.tensor_add(out=y3p, in0=y3p, in1=b3t)
                y3 = apool.tile([BLK, 1], F32, name="y3")
                nc.scalar.activation(out=y3, in_=y3p,
                                     func=mybir.ActivationFunctionType.Sigmoid)
                nc.sync.dma_start(out=y_d[osl], in_=y3)

    return nc


def build_bass(variant: str = "full", repeats: int = 1) -> bass.Bass:
    """variant: 'full' | 'dma_only' (skip per-feature compute) |
    'dve_only' (load each W chunk once, keep all compute ops).
    repeats: unroll the whole computation R times inside one NEFF (device-
    side timing: kernel_time = (T(R2)-T(R1))/(R2-R1))."""
    if variant == "split":
        return build_bass_split(repeats)
    dma_only = variant == "dma_only"
    dve_only = variant == "dve_only"
    noop = variant == "noop"
    nc = bass.Bass("TRN2", target_bir_lowering=False, debug=False,
                   num_devices=N_CORES)

    x_d = nc.dram_tensor("x", [O_PER_CORE, IN_DIM], F32, kind="ExternalInput").ap()
    w1_d = nc.dram_tensor("W1", [O_PER_CORE, MID, IN_DIM], F32, kind="ExternalInput").ap()
    b1_d = nc.dram_tensor("b1", [O_PER_CORE, MID], F32, kind="ExternalInput").ap()
    w2_d = nc.dram_tensor("W2", [O_PER_CORE, MID, MID], F32, kind="ExternalInput").ap()
    b2_d = nc.dram_tensor("b2", [O_PER_CORE, MID], F32, kind="ExternalInput").ap()
    w3_d = nc.dram_tensor("W3", [O_PER_CORE, 1, MID], F32, kind="ExternalInput").ap()
    b3_d = nc.dram_tensor("b3", [O_PER_CORE, 1], F32, kind="ExternalInput").ap()
    y_d = nc.dram_tensor("y", [O_PER_CORE], F32, kind="ExternalOutput").ap()

    if noop:
        with tile.TileContext(nc) as tc:
            with tc.tile_pool(name="np_", bufs=1) as pool:
                for b in range(N_BLK):
                    osl = slice(b * BLK, (b + 1) * BLK)
                    b3t = pool.tile([BLK, 1], F32, name="b3t")
                    nc.sync.dma_start(out=b3t, in_=b3_d[osl, :])
                    y3 = pool.tile([BLK, 1], F32, name="y3")
                    nc.scalar.activation(
                        out=y3, in_=b3t,
                        func=mybir.ActivationFunctionType.Sigmoid)
                    nc.sync.dma_start(out=y_d[osl], in_=y3)
        return nc

    with tile.TileContext(nc) as tc:
        with (
            tc.tile_pool(name="wpool", bufs=2) as wpool,
            tc.tile_pool(name="apool", bufs=2) as apool,
            tc.tile_pool(name="spool", bufs=2) as spool,
        ):
          with (tc.For_i(0, repeats, 1) if repeats > 1
                else _nullctx()):
            for b in range(N_BLK):
                osl = slice(b * BLK, (b + 1) * BLK)

                xt = apool.tile([BLK, IN_DIM], F32, name="xt")
                nc.sync.dma_start(out=xt, in_=x_d[osl, :])
                b1t = apool.tile([BLK, MID], F32, name="b1t")
                nc.sync.dma_start(out=b1t, in_=b1_d[osl, :])
                b2t = apool.tile([BLK, MID], F32, name="b2t")
                nc.sync.dma_start(out=b2t, in_=b2_d[osl, :])
                b3t = apool.tile([BLK, 1], F32, name="b3t")
                nc.sync.dma_start(out=b3t, in_=b3_d[osl, :])
                w3t = apool.tile([BLK, MID], F32, name="w3t")
                nc.sync.dma_start(out=w3t, in_=w3_d[osl, 0, :])

                y1p = apool.tile([BLK, MID], F32, name="y1p")
                y1 = apool.tile([BLK, MID], F32, name="y1")
                y2p = apool.tile([BLK, MID], F32, name="y2p")
                y2 = apool.tile([BLK, MID], F32, name="y2")
                dum1 = spool.tile([BLK, 1], F32, name="dum1", tag="dum1")
                dum2 = spool.tile([BLK, 1], F32, name="dum2", tag="dum2")

                # ---- layer 1: y1[o, m] = b1[o, m] + sum_i W1[o, m, i] x[o, i]
                n_w1_loads = 1 if dve_only else MID // M_CHUNK
                w1cs = []
                for mc in range(n_w1_loads):
                    w1c = wpool.tile([BLK, M_CHUNK, IN_DIM], F32, name="w1c",
                                     tag="w1c")
                    nc.sync.dma_start(
                        out=w1c,
                        in_=w1_d[osl, mc * M_CHUNK:(mc + 1) * M_CHUNK, :])
                    w1cs.append(w1c)
                if not dma_only:
                    for mc in range(MID // M_CHUNK):
                        w1c = w1cs[min(mc, n_w1_loads - 1)]
                        for m in range(M_CHUNK):
                            mm = mc * M_CHUNK + m
                            nc.vector.scalar_tensor_tensor(
                                out=dum1.broadcast_to((BLK, IN_DIM)),
                                in0=w1c[:, m, :],
                                scalar=1.0,
                                in1=xt,
                                op0=mybir.AluOpType.mult,
                                op1=mybir.AluOpType.mult,
                                accum_out=y1p[:, mm:mm + 1],
                            )
                    nc.vector.tensor_add(out=y1, in0=y1p, in1=b1t)

                # ---- layer 2: y2[o, n] = sigmoid(b2 + sum_m W2[o, n, m] y1[o, m])
                n_w2_loads = 1 if dve_only else MID // N_CHUNK
                w2cs = []
                for nc_i in range(n_w2_loads):
                    w2c = wpool.tile([BLK, N_CHUNK, MID], F32, name="w2c",
                                     tag="w2c")
                    nc.sync.dma_start(
                        out=w2c,
                        in_=w2_d[osl, nc_i * N_CHUNK:(nc_i + 1) * N_CHUNK, :])
                    w2cs.append(w2c)
                if not dma_only:
                    for nc_i in range(MID // N_CHUNK):
                        w2c = w2cs[min(nc_i, n_w2_loads - 1)]
                        for n in range(N_CHUNK):
                            nn = nc_i * N_CHUNK + n
                            nc.vector.scalar_tensor_tensor(
                                out=dum2.broadcast_to((BLK, MID)),
                                in0=w2c[:, n, :],
                                scalar=1.0,
                                in1=y1,
                                op0=mybir.AluOpType.mult,
                                op1=mybir.AluOpType.mult,
                                accum_out=y2p[:, nn:nn + 1],
                            )
                    nc.vector.tensor_add(out=y2p, in0=y2p, in1=b2t)
                    nc.scalar.activation(out=y2, in_=y2p,
                                         func=mybir.ActivationFunctionType.Sigmoid)

                # ---- layer 3: y[o] = sigmoid(b3 + sum_n W3[o, 0, n] y2[o, n])
                dum3 = spool.tile([BLK, 1], F32, name="dum3", tag="dum3")
                y3p = apool.tile([BLK, 1], F32, name="y3p")
                if dma_only:
                    nc.vector.tensor_add(out=y3p, in0=b3t, in1=b3t)
                else:
                    nc.vector.scalar_tensor_tensor(
                        out=dum3.broadcast_to((BLK, MID)),
                        in0=w3t,
                        scalar=1.0,
                        in1=y2,
                        op0=mybir.AluOpType.mult,
                        op1=mybir.AluOpType.mult,
                        accum_out=y3p,
                    )
                    nc.vector.tensor_add(out=y3p, in0=y3p, in1=b3t)
                y3 = apool.tile([BLK, 1], F32, name="y3")
                nc.scalar.activation(out=y3, in_=y3p,
                                     func=mybir.ActivationFunctionType.Sigmoid)
                nc.sync.dma_start(out=y_d[osl], in_=y3)

    return nc


_NC_CACHE = {}


def _get_nc(variant: str = "full", repeats: int = 1):
    key = (variant, repeats)
    if key not in _NC_CACHE:
        _NC_CACHE[key] = build_bass(variant, repeats)
    return _NC_CACHE[key]


def _shard_inputs(inputs: dict) -> list[dict]:
    arrs = {k: np.ascontiguousarray(np.asarray(v, dtype=np.float32))
            for k, v in inputs.items()}
    in_maps = []
    for c in range(N_CORES):
        sl = slice(c * O_PER_CORE, (c + 1) * O_PER_CORE)
        in_maps.append({k: np.ascontiguousarray(v[sl]) for k, v in arrs.items()})
    return in_maps


def run(inputs: dict, trace: bool = False):
    """Run the SPMD kernel; returns (y_full, BassKernelResults)."""
    nc = _get_nc()
    in_maps = _shard_inputs(inputs)
    res = run_bass_kernel_spmd(nc, in_maps, core_ids=list(range(N_CORES)),
                               trace=trace)
    y = np.concatenate([r["y"] for r in res.results])
    return y, res


def kernel(**inputs) -> np.ndarray:
    y, _ = run(inputs, trace=False)
    return y


# ---------------------------------------------------------------------------
# Dev-only timing helper: chain k dependent executions of the NEFF inside one
# jit so per-call RPC/dispatch overhead amortizes; per-iteration time
# ~= (T(k) - T(1)) / (k - 1).  (No NTFF profiling is available under axon in
# this container.)
# ---------------------------------------------------------------------------

def _build_chained_fn(nc, n_cores: int, k: int):
    import jax
    from jax.sharding import Mesh, PartitionSpec
    try:
        from jax.experimental.shard_map import shard_map
    except ImportError:
        from jax.sharding import shard_map
    from concourse.bass2jax import (
        _bass_exec_p, install_neuronx_cc_hook, partition_id_tensor,
    )

    install_neuronx_cc_hook()
    partition_name = nc.partition_id_tensor.name if nc.partition_id_tensor else None

    in_names, out_names, out_avals, zero_outs = [], [], [], []
    for alloc in nc.m.functions[0].allocations:
        if not isinstance(alloc, mybir.MemoryLocationSet):
            continue
        name = alloc.memorylocations[0].name
        if alloc.kind == "ExternalInput":
            if name != partition_name:
                in_names.append(name)
        elif alloc.kind == "ExternalOutput":
            shape = tuple(alloc.tensor_shape)
            dtype = mybir.dt.np(alloc.dtype)
            out_names.append(name)
            out_avals.append(jax.core.ShapedArray(shape, dtype))
            zero_outs.append(np.zeros(shape, dtype))
    n_params = len(in_names)
    n_outs = len(out_avals)
    x_idx = in_names.index("x")
    bind_in_names = tuple(in_names + out_names
                          + ([partition_name] if partition_name else []))

    del k, x_idx

    def _body(*args):
        ins = list(args[:n_params])
        zeros = list(args[n_params:n_params + n_outs])
        operands = ins + zeros
        if partition_name is not None:
            operands.append(partition_id_tensor())
        outs = _bass_exec_p.bind(
            *operands,
            out_avals=tuple(out_avals),
            in_names=bind_in_names,
            out_names=tuple(out_names),
            lowering_input_output_aliases=(),
            sim_require_finite=True,
            sim_require_nnan=True,
            nc=nc,
        )
        return tuple(outs)

    devices = jax.devices()[:n_cores]
    mesh = Mesh(np.asarray(devices), ("core",))
    in_specs = (PartitionSpec("core"),) * (n_params + n_outs)
    out_specs = (PartitionSpec("core"),) * n_outs
    fn = jax.jit(shard_map(_body, mesh=mesh, in_specs=in_specs,
                           out_specs=out_specs, check_rep=False))
    return fn, mesh, in_names, zero_outs, n_params


def time_chained(inputs: dict, k: int, reps: int = 5, variant: str = "full",
                 repeats: int = 1):
    """Return min wall seconds for k pipelined kernel executions."""
    import time as _time

    import jax
    from jax.sharding import NamedSharding, PartitionSpec

    nc = _get_nc(variant, repeats)
    in_maps = _shard_inputs(inputs)
    fn, mesh, in_names, zero_outs, n_params = _build_chained_fn(nc, N_CORES, k)
    sh = NamedSharding(mesh, PartitionSpec("core"))
    concat_in = [
        jax.device_put(
            np.concatenate([m[name] for m in in_maps], axis=0), sh)
        for name in in_names
    ]
    concat_zeros = [
        jax.device_put(
            np.zeros((N_CORES * z.shape[0], *z.shape[1:]), z.dtype), sh)
        for z in zero_outs
    ]
    args = concat_in + concat_zeros
    out = fn(*args)
    jax.block_until_ready(out)  # compile + warm
    best = float("inf")
    for _ in range(reps):
        t0 = _time.perf_counter()
        outs = [fn(*args) for _ in range(k)]
        jax.block_until_ready(outs)
        best = min(best, _time.perf_counter() - t0)
    return best, np.asarray(outs[-1][0])

